# revision 35
# baseline (speedup 1.0000x reference)
"""Trainium2 Bass kernel for nn_MultiHeadAttention_79706003079680.

Reference (fp32):
    qp = (q @ Wq + bq) * SCALE      # [B, N, PROJ]
    kp = k @ Wk + bk
    vp = v @ Wv + bv
    scores = einsum('bnd,bmd->bnm', qp, kp)
    attn = softmax(scores, axis=1)          # over the QUERY axis n
    x = einsum('bnm,bmd->bnd', attn, vp)
    out = x @ Wo + bo                       # [B, N, HIDDEN]

Algebraic collapse (the PROJ=4096 dim never materializes):
    scores = q @ A @ k^T (+ per-m terms that cancel under softmax over n,
                          + per-n term q @ (SCALE*Wq@bk) -- requires bk == 0,
                          asserted host-side; bq cancels EXACTLY for any bq)
      with A   = SCALE * Wq @ Wk^T          # [512, 512], host-precomputed
    out = attn @ (v @ Wvo + bv@Wo) + bo
      with Wvo = Wv @ Wo                    # [512, 512], host-precomputed

Per-core MACs drop ~10x vs the reference shape (51.5e9 -> 5.4e9 per batch).

Sharding: 8 cores = 4 batches x 2 key-halves (m in [mh*1024, mh*1024+1024)).
softmax(axis=n) is per-key-m, so an m-split needs no cross-core coupling.
Each core emits a partial out^T [HIDDEN, N]; the host sums the two
key-halves per batch, transposes, and adds bo (same as before).

On-chip flow (per core, P=128 partitions, everything SBUF-resident):
  qT  [128, 4, 2048] fp16   via DMA-xbar transpose (4 chunk tiles)
  kT  [128, 4, 1024] fp16   via DMA-xbar transpose
  vT  [128, 4, 1024] fp16   via DMA-xbar transpose
  kaT = A^T k^T [128, 4, 1024] fp16  (the m-side of the bilinear form is
        half the size of the q-side, so associate scores = q (A k^T))
  scoresT[m, n] in PSUM megatiles [128, 2048] f32, contraction over h,
        lhsT = kaT blocks, rhs = raw qT chunks
  attnT = exp(scoresT) bf16 (UNNORMALIZED; rowsum via activation accum)
  vpo_s = (v @ Wvo + bvo) * (1/rowsum)  bf16  (per-partition scale = per-m)
  outT += vpo_s^T-blocks @ attnT        (normalization folded into vpo_s)

All DMAs ride the SP (nc.sync) HWDGE queue: DMAs issued from the
Activation queue returned corrupted data on this hardware path.
All PSUM phases allocate [128, 2048] megatiles from ONE pool (slices as
banks) so no pool-transition barriers serialize the phases.
"""

import numpy as np

import concourse.bass as bass
import concourse.mybir as mybir
import concourse.tile as tile

P = 128
HIDDEN = 512
B, N = 4, 2048
M = N // 2                         # keys per core = 1024
SCALE = (HIDDEN // 8) ** -0.5      # att_size ** -0.5 = 64 ** -0.5

HB = HIDDEN // P                   # 4 h-blocks of 128
NCH = N // 512                     # 4 n-chunks of 512
MB = M // P                        # 8 m-blocks of 128

F32 = mybir.dt.float32
F16 = mybir.dt.float16
BF16 = mybir.dt.bfloat16
AF = mybir.ActivationFunctionType


MAX_WAITS = 1


def split_excess_waits(nc, max_waits=MAX_WAITS):
    """Move excess per-instruction sem waits onto same-engine NoOps.

    This walrus build rejects instructions carrying more than a couple of
    sync-wait commands ("Too many sync wait commands" in setupSyncWait).
    A NoOp placed immediately before the instruction on the same engine
    enforces the wait in program order with identical semantics.
    """
    n_extra = 0
    for f in nc.m.functions:
        for bb in f.blocks:
            insts = bb.instructions
            i = 0
            while i < len(insts):
                inst = insts[i]
                si = getattr(inst, "sync_info", None)
                if si is not None and si.on_wait and len(si.on_wait) > max_waits:
                    waits = list(si.on_wait)
                    si.on_wait = waits[: max_waits]
                    for w in waits[max_waits:]:
                        n_extra += 1
                        nop = mybir.InstNoOp(
                            name=f"I-wsplit{n_extra}",
                            ins=[],
                            outs=[],
                            engine=inst.engine,
                        )
                        nop.sync_info = mybir.SyncInfo(on_wait=[w], on_update=[])
                        try:
                            nc.register_instruction(nop)
                        except Exception:
                            pass
                        # insert immediately before inst (inst shifts right)
                        insts.insert(i, nop)
                        i += 1
                i += 1
    return n_extra


class PatchedTC(tile.TileContext):
    """TileContext that post-processes the module to satisfy this walrus
    build's per-instruction sync-wait limit."""

    def __exit__(self, exc_type, exc_val, exc_tb):
        ret = super().__exit__(exc_type, exc_val, exc_tb)
        if exc_type is None:
            split_excess_waits(self.nc)
        return ret


def build_nc(include_bvo=False):
    nc = bass.Bass("TRN2", target_bir_lowering=False, debug=False, num_devices=8)

    qT16 = nc.dram_tensor("qT16", [HIDDEN, N], F16, kind="ExternalInput")
    kT16 = nc.dram_tensor("kT16", [HIDDEN, M], F16, kind="ExternalInput")
    vT16 = nc.dram_tensor("vT16", [HIDDEN, M], F16, kind="ExternalInput")
    AT16 = nc.dram_tensor("AT16", [HIDDEN, HIDDEN], F16, kind="ExternalInput")
    W16 = nc.dram_tensor("W16", [HIDDEN, HIDDEN], F16, kind="ExternalInput")
    if include_bvo:
        bvo16 = nc.dram_tensor("bvo16", [HIDDEN], F16, kind="ExternalInput")
    outT = nc.dram_tensor("outT", [HIDDEN, N], F16, kind="ExternalOutput")

    qT_v = qT16.ap().rearrange("(hb p) n -> p hb n", p=P)
    kT_v = kT16.ap().rearrange("(hb p) m -> p hb m", p=P)
    vT_v = vT16.ap().rearrange("(hb p) m -> p hb m", p=P)

    outT_v = outT.ap().rearrange("(hb p) n -> p hb n", p=P)

    with PatchedTC(nc) as tc:
        with (
            tc.tile_pool(name="singles", bufs=1) as singles,
            tc.tile_pool(name="acts", bufs=1) as acts,
            tc.tile_pool(name="ostg", bufs=2) as ostg,
        ):
            # --- loads. q/k/v arrive HOST-pre-transposed, so everything is
            # a plain DMACopy. The SP (nc.sync) queue carries the kA/scores
            # critical path in engine order (AT, k chunks, q chunks); the
            # Activation queue carries the rest (plain DMAs there are safe;
            # xbar transposes there returned corrupted data on HW).
            # AT is chunked by output h-block and interleaved with the k
            # chunks so the first kA accumulation chain (needs AT[:, :,
            # 0:128] + k chunk 0 only) starts ~2.5us in.
            AT_v = AT16.ap().rearrange("(hpb p) h -> p hpb h", p=P)
            AT_s = acts.tile([P, HB, HIDDEN], F16, tag="AT")
            nc.sync.dma_start(out=AT_s[:, :, 0:P], in_=AT_v[:, :, 0:P])
            kTc = []
            for mch in range(M // 512):
                t = acts.tile([P, HB, 512], F16, tag=f"kT{mch}")
                nc.sync.dma_start(
                    out=t, in_=kT_v[:, :, mch * 512 : (mch + 1) * 512]
                )
                kTc.append(t)
            nc.sync.dma_start(out=AT_s[:, :, P:], in_=AT_v[:, :, P:])
            qTc = []
            for nch in range(NCH):
                t = acts.tile([P, HB, 512], F16, tag=f"qT{nch}")
                nc.sync.dma_start(
                    out=t, in_=qT_v[:, :, nch * 512 : (nch + 1) * 512]
                )
                qTc.append(t)
            # W and vT are consumed late (scores end / vpo); keep them on
            # the SAME queue BEHIND the critical chain — a second queue's
            # issues race ahead and steal the serialized DMA engine.
            W_s = acts.tile([P, HB, HIDDEN], F16, tag="W")
            nc.sync.dma_start(
                out=W_s, in_=W16.ap().rearrange("(hb p) h -> p hb h", p=P)
            )
            vT = acts.tile([P, HB, M], F16, tag="vT")
            nc.sync.dma_start(out=vT, in_=vT_v)
            if include_bvo:
                bvo_row = singles.tile([1, HIDDEN], F16)
                nc.scalar.dma_start(
                    out=bvo_row, in_=bvo16.ap().rearrange("(o a) -> o a", o=1)
                )
                ones_tmp = singles.tile([1, P], F16)
                nc.vector.memset(ones_tmp, 1.0)
                ones_row = singles.tile([1, P], F16)
                nc.vector.tensor_copy(ones_row, ones_tmp)
            rsum = singles.tile([P, MB], F32)
            rinv = singles.tile([P, MB], F32)

            kaT = acts.tile([P, HB, M], F16, tag="kaT")
            attnT = acts.tile([P, MB, N], BF16, tag="attnT")
            vpo_s = acts.tile([P, MB, HIDDEN], BF16, tag="vpo")

            with tc.tile_pool(name="ps", bufs=2, space="PSUM") as psp:
                # --- kaT[h, m] = sum_h' A[h, h'] k[m, h']  (lhsT = A^T) ---
                # The m-side of the bilinear score form is half the q-side,
                # so fold A into k rather than into q.
                for hb in range(HB):
                    mega = psp.tile([P, N], F32, tag="mega", name=f"ka{hb}")
                    for hpb in range(HB):
                        w = AT_s[:, hpb, hb * P : (hb + 1) * P]
                        for mch in range(M // 512):
                            nc.tensor.matmul(
                                mega[:, mch * 512 : (mch + 1) * 512],
                                w,
                                kTc[mch][:, hpb, :],
                                start=(hpb == 0),
                                stop=(hpb == HB - 1),
                            )
                    for mch in range(M // 512):
                        if mch % 2 == 0:
                            nc.scalar.activation(
                                kaT[:, hb, mch * 512 : (mch + 1) * 512],
                                mega[:, mch * 512 : (mch + 1) * 512],
                                AF.Identity,
                            )
                        else:
                            nc.vector.tensor_copy(
                                kaT[:, hb, mch * 512 : (mch + 1) * 512],
                                mega[:, mch * 512 : (mch + 1) * 512],
                            )

                # --- scoresT + softmax(exp, unnormalized) per m-block ----
                # scoresT[m, n] = sum_h kaT[h, m] * qT[h, n]
                for mb in range(MB):
                    sc = psp.tile([P, N], F32, tag="mega", name=f"sc{mb}")
                    for hb in range(HB):
                        w = kaT[:, hb, mb * P : (mb + 1) * P]
                        for nch in range(NCH):
                            nc.tensor.matmul(
                                sc[:, nch * 512 : (nch + 1) * 512],
                                w,
                                qTc[nch][:, hb, :],
                                start=(hb == 0),
                                stop=(hb == HB - 1),
                            )
                    # e = exp(scores); rowsum via accumulator. No max-shift:
                    # |scores| < ~60 and exp(60) is comfortably finite in f32.
                    nc.scalar.activation(
                        attnT[:, mb, :],
                        sc,
                        AF.Exp,
                        accum_out=rsum[:, mb : mb + 1],
                    )
                    nc.vector.reciprocal(rinv[:, mb : mb + 1], rsum[:, mb : mb + 1])

                # --- vpo_s = (v @ Wvo + bvo) * rinv (normalization folded)
                # one megatile hosts FOUR [128,512] vpo banks, so the two
                # pool slots cover all 8 m-blocks with zero PSUM reuse stalls
                for quad in range(MB // 4):
                    mega = psp.tile([P, N], F32, tag="mega", name=f"vp{quad}")
                    for sub in range(4):
                        mb = 4 * quad + sub
                        ps = mega[:, sub * 512 : (sub + 1) * 512]
                        for hb in range(HB):
                            nc.tensor.matmul(
                                ps,
                                vT[:, hb, mb * P : (mb + 1) * P],
                                W_s[:, hb, :],
                                start=(hb == 0),
                                stop=(not include_bvo and hb == HB - 1),
                            )
                        if include_bvo:
                            nc.tensor.matmul(
                                ps, ones_row, bvo_row, start=False, stop=True
                            )
                        nc.scalar.activation(
                            vpo_s[:, mb, :],
                            ps,
                            AF.Identity,
                            scale=rinv[:, mb : mb + 1],
                        )

                # --- outT[h, n] = sum_m vpo_s[m, h] * attnT[m, n] --------
                for nch in range(NCH):
                    mega = psp.tile([P, N], F32, tag="mega", name=f"ob{nch}")
                    for mb in range(MB):
                        for hb in range(HB):
                            nc.tensor.matmul(
                                mega[:, hb * 512 : (hb + 1) * 512],
                                vpo_s[:, mb, hb * P : (hb + 1) * P],
                                attnT[:, mb, nch * 512 : (nch + 1) * 512],
                                start=(mb == 0),
                                stop=(mb == MB - 1),
                            )
                    # two half-DMAs per chunk: the first half's transfer
                    # overlaps the second half's PSUM->SBUF copies
                    st = ostg.tile([P, HB, 512], F16, tag="st")
                    for half in range(2):
                        for hb in (2 * half, 2 * half + 1):
                            if hb % 2 == 0:
                                nc.vector.tensor_copy(
                                    st[:, hb, :], mega[:, hb * 512 : (hb + 1) * 512]
                                )
                            else:
                                nc.scalar.activation(
                                    st[:, hb, :],
                                    mega[:, hb * 512 : (hb + 1) * 512],
                                    AF.Identity,
                                )
                        nc.sync.dma_start(
                            out=outT_v[
                                :,
                                2 * half : 2 * half + 2,
                                nch * 512 : (nch + 1) * 512,
                            ],
                            in_=st[:, 2 * half : 2 * half + 2, :],
                        )
    # A handful of waits are attached after the TileContext's own exit
    # processing; sweep again until the module is clean.
    while split_excess_waits(nc):
        pass
    return nc


class _Runner:
    """Compile the Bass program once; re-execute cheaply on later calls.

    Mirrors bass2jax.run_bass_via_pjrt's multi-core path, but keeps the
    jitted shard_map callable so repeated kernel() calls skip the
    multi-minute neuronxcc compile.
    """

    def __init__(self, include_bvo=False):
        import jax
        from jax.sharding import Mesh, PartitionSpec
        from jax.experimental.shard_map import shard_map
        from concourse import bass2jax
        import concourse.mybir as mb

        self.jax = jax
        nc = build_nc(include_bvo=include_bvo)
        self.nc = nc
        bass2jax.install_neuronx_cc_hook()

        in_names, out_names, out_avals, zero_outs = [], [], [], []
        partition_name = (
            nc.partition_id_tensor.name if nc.partition_id_tensor else None
        )
        for alloc in nc.m.functions[0].allocations:
            if not isinstance(alloc, mb.MemoryLocationSet):
                continue
            name = alloc.memorylocations[0].name
            if alloc.kind == "ExternalInput":
                if name != partition_name:
                    in_names.append(name)
            elif alloc.kind == "ExternalOutput":
                shape = tuple(alloc.tensor_shape)
                dtype = mb.dt.np(alloc.dtype)
                out_names.append(name)
                out_avals.append(jax.core.ShapedArray(shape, dtype))
                zero_outs.append(np.zeros(shape, dtype))
        n_params = len(in_names)
        n_outs = len(out_avals)
        all_in_names = list(in_names) + list(out_names)
        if partition_name is not None:
            all_in_names.append(partition_name)
        self.in_names = in_names
        self.out_names = out_names
        self.zero_outs = zero_outs

        def _body(*args):
            operands = list(args)
            if partition_name is not None:
                operands.append(bass2jax.partition_id_tensor())
            outs = bass2jax._bass_exec_p.bind(
                *operands,
                out_avals=tuple(out_avals),
                in_names=tuple(all_in_names),
                out_names=tuple(out_names),
                lowering_input_output_aliases=(),
                sim_require_finite=True,
                sim_require_nnan=True,
                nc=nc,
            )
            return tuple(outs)

        devices = jax.devices()[:8]
        mesh = Mesh(np.asarray(devices), ("core",))
        self.mesh = mesh
        in_specs = (PartitionSpec("core"),) * (n_params + n_outs)
        out_specs = (PartitionSpec("core"),) * n_outs
        self.body = _body
        self.in_specs = in_specs
        self.out_specs = out_specs
        donate = tuple(range(n_params, n_params + n_outs))
        self.sharded = jax.jit(
            shard_map(
                _body,
                mesh=mesh,
                in_specs=in_specs,
                out_specs=out_specs,
                check_rep=False,
            ),
            donate_argnums=donate,
            keep_unused=True,
        )
        self.out_avals = out_avals

    def prepare(self, in_maps):
        """Concatenate per-core inputs along axis 0 (device-shardable)."""
        return [
            np.concatenate([in_maps[c][name] for c in range(8)], axis=0)
            for name in self.in_names
        ]

    def run(self, concat_in):
        zeros = [
            np.zeros((8 * z.shape[0], *z.shape[1:]), z.dtype) for z in self.zero_outs
        ]
        out_arrs = self.sharded(*concat_in, *zeros)
        res = []
        for c in range(8):
            res.append(
                {
                    name: np.asarray(out_arrs[i]).reshape(
                        8, *self.out_avals[i].shape
                    )[c]
                    for i, name in enumerate(self.out_names)
                }
            )
        return res


_RUNNERS = {}


def _get_runner(include_bvo=False):
    if include_bvo not in _RUNNERS:
        _RUNNERS[include_bvo] = _Runner(include_bvo=include_bvo)
    return _RUNNERS[include_bvo]


def make_in_maps(inputs):
    f32 = lambda x: np.asarray(x, dtype=np.float32)
    q, k, v = f32(inputs["q"]), f32(inputs["k"]), f32(inputs["v"])
    Wq, Wk, Wv, Wo = (f32(inputs[n]) for n in ("Wq", "Wk", "Wv", "Wo"))
    bk, bv = f32(inputs["bk"]), f32(inputs["bv"])
    if np.any(bk != 0):
        # A nonzero bk adds a per-query term q @ (SCALE*Wq@bk) to the
        # scores which this kernel does not materialize. (bq-only terms
        # cancel exactly under the softmax over queries; bv/bo are handled.)
        raise NotImplementedError("nonzero bk is not supported by this kernel")
    AT16 = np.ascontiguousarray((SCALE * (Wk @ Wq.T)).astype(np.float16))
    W16 = np.ascontiguousarray((Wv @ Wo).astype(np.float16))
    bvo16 = np.ascontiguousarray((bv @ Wo).astype(np.float16))
    # host-side fp16 cast AND transpose: the device consumes q/k/v with the
    # hidden dim on partitions, so ship them pre-transposed and every load
    # becomes a plain contiguous-ish DMACopy (no xbar transposes).
    q16 = q.astype(np.float16)
    k16 = k.astype(np.float16)
    v16 = v.astype(np.float16)
    in_maps = []
    for c in range(8):
        b, mh = c // 2, c % 2
        sl = slice(mh * M, (mh + 1) * M)
        in_maps.append(
            {
                "qT16": np.ascontiguousarray(q16[b].T),
                "kT16": np.ascontiguousarray(k16[b, sl].T),
                "vT16": np.ascontiguousarray(v16[b, sl].T),
                "AT16": AT16,
                "W16": W16,
                "bvo16": bvo16,
            }
        )
    return in_maps


def assemble_out(results, bo):
    out = np.empty((B, N, HIDDEN), dtype=np.float32)
    for b in range(B):
        acc = results[2 * b]["outT"].astype(np.float32) + results[2 * b + 1]["outT"]
        out[b] = acc.T + bo[None, :]
    return out


def kernel(**inputs):
    in_maps = make_in_maps(inputs)
    include_bvo = bool(np.any(in_maps[0]["bvo16"] != 0))
    runner = _get_runner(include_bvo=include_bvo)
    res = runner.run(runner.prepare(in_maps))
    bo = np.asarray(inputs["bo"], dtype=np.float32)
    return assemble_out(res, bo)


# revision 38
# speedup vs baseline: 1.3097x; 1.3097x over previous
"""Trainium2 Bass kernel for nn_MultiHeadAttention_79706003079680.

Reference (fp32):
    qp = (q @ Wq + bq) * SCALE      # [B, N, PROJ]
    kp = k @ Wk + bk
    vp = v @ Wv + bv
    scores = einsum('bnd,bmd->bnm', qp, kp)
    attn = softmax(scores, axis=1)          # over the QUERY axis n
    x = einsum('bnm,bmd->bnd', attn, vp)
    out = x @ Wo + bo                       # [B, N, HIDDEN]

Algebraic collapse (the PROJ=4096 dim never materializes):
    scores = q @ A @ k^T (+ per-m terms that cancel under softmax over n,
                          + per-n term q @ (SCALE*Wq@bk) -- requires bk == 0,
                          asserted host-side; bq cancels EXACTLY for any bq)
      with A   = SCALE * Wq @ Wk^T          # [512, 512], host-precomputed
    out = attn @ (v @ Wvo + bv@Wo) + bo
      with Wvo = Wv @ Wo                    # [512, 512], host-precomputed

Per-core MACs drop ~10x vs the reference shape (51.5e9 -> 5.4e9 per batch).

Sharding: 8 cores = 4 batches x 2 key-halves (m in [mh*1024, mh*1024+1024)).
softmax(axis=n) is per-key-m, so an m-split needs no cross-core coupling.
Each core emits a partial out^T [HIDDEN, N]; the host sums the two
key-halves per batch, transposes, and adds bo (same as before).

On-chip flow (per core, P=128 partitions, everything SBUF-resident):
  qT  [128, 4, 2048] fp16   via DMA-xbar transpose (4 chunk tiles)
  kT  [128, 4, 1024] fp16   via DMA-xbar transpose
  vT  [128, 4, 1024] fp16   via DMA-xbar transpose
  kaT = A^T k^T [128, 4, 1024] fp16  (the m-side of the bilinear form is
        half the size of the q-side, so associate scores = q (A k^T))
  scoresT[m, n] in PSUM megatiles [128, 2048] f32, contraction over h,
        lhsT = kaT blocks, rhs = raw qT chunks
  attnT = exp(scoresT) bf16 (UNNORMALIZED; rowsum via activation accum)
  vpo_s = (v @ Wvo + bvo) * (1/rowsum)  bf16  (per-partition scale = per-m)
  outT += vpo_s^T-blocks @ attnT        (normalization folded into vpo_s)

All DMAs ride the SP (nc.sync) HWDGE queue: DMAs issued from the
Activation queue returned corrupted data on this hardware path.
All PSUM phases allocate [128, 2048] megatiles from ONE pool (slices as
banks) so no pool-transition barriers serialize the phases.
"""

import numpy as np

import concourse.bass as bass
import concourse.mybir as mybir
import concourse.tile as tile

P = 128
HIDDEN = 512
B, N = 4, 2048
M = N // 2                         # keys per core = 1024
SCALE = (HIDDEN // 8) ** -0.5      # att_size ** -0.5 = 64 ** -0.5

HB = HIDDEN // P                   # 4 h-blocks of 128
NCH = N // 512                     # 4 n-chunks of 512
MB = M // P                        # 8 m-blocks of 128

F32 = mybir.dt.float32
F16 = mybir.dt.float16
BF16 = mybir.dt.bfloat16
AF = mybir.ActivationFunctionType


MAX_WAITS = 1


def split_excess_waits(nc, max_waits=MAX_WAITS):
    """Move excess per-instruction sem waits onto same-engine NoOps.

    This walrus build rejects instructions carrying more than a couple of
    sync-wait commands ("Too many sync wait commands" in setupSyncWait).
    A NoOp placed immediately before the instruction on the same engine
    enforces the wait in program order with identical semantics.
    """
    n_extra = 0
    for f in nc.m.functions:
        for bb in f.blocks:
            insts = bb.instructions
            i = 0
            while i < len(insts):
                inst = insts[i]
                si = getattr(inst, "sync_info", None)
                if si is not None and si.on_wait and len(si.on_wait) > max_waits:
                    waits = list(si.on_wait)
                    si.on_wait = waits[: max_waits]
                    for w in waits[max_waits:]:
                        n_extra += 1
                        nop = mybir.InstNoOp(
                            name=f"I-wsplit{n_extra}",
                            ins=[],
                            outs=[],
                            engine=inst.engine,
                        )
                        nop.sync_info = mybir.SyncInfo(on_wait=[w], on_update=[])
                        try:
                            nc.register_instruction(nop)
                        except Exception:
                            pass
                        # insert immediately before inst (inst shifts right)
                        insts.insert(i, nop)
                        i += 1
                i += 1
    return n_extra


class PatchedTC(tile.TileContext):
    """TileContext that post-processes the module to satisfy this walrus
    build's per-instruction sync-wait limit."""

    def __exit__(self, exc_type, exc_val, exc_tb):
        ret = super().__exit__(exc_type, exc_val, exc_tb)
        if exc_type is None:
            split_excess_waits(self.nc)
        return ret


def build_nc(include_bvo=False):
    nc = bass.Bass("TRN2", target_bir_lowering=False, debug=False, num_devices=8)

    qT16 = nc.dram_tensor("qT16", [HIDDEN, N], F16, kind="ExternalInput")
    kT16 = nc.dram_tensor("kT16", [HIDDEN, M], F16, kind="ExternalInput")
    vT16 = nc.dram_tensor("vT16", [HIDDEN, M], F16, kind="ExternalInput")
    AT16 = nc.dram_tensor("AT16", [HIDDEN, HIDDEN], F16, kind="ExternalInput")
    W16 = nc.dram_tensor("W16", [HIDDEN, HIDDEN], F16, kind="ExternalInput")
    if include_bvo:
        bvo16 = nc.dram_tensor("bvo16", [HIDDEN], F16, kind="ExternalInput")
    outT = nc.dram_tensor("outT", [HIDDEN, N], F16, kind="ExternalOutput")

    qT_v = qT16.ap().rearrange("(hb p) n -> p hb n", p=P)
    kT_v = kT16.ap().rearrange("(hb p) m -> p hb m", p=P)
    vT_v = vT16.ap().rearrange("(hb p) m -> p hb m", p=P)

    outT_v = outT.ap().rearrange("(hb p) n -> p hb n", p=P)

    with PatchedTC(nc) as tc:
        with (
            tc.tile_pool(name="singles", bufs=1) as singles,
            tc.tile_pool(name="acts", bufs=1) as acts,
            tc.tile_pool(name="ostg", bufs=2) as ostg,
        ):
            # --- loads. q/k/v arrive HOST-pre-transposed, so everything is
            # a plain DMACopy. The SP (nc.sync) queue carries the kA/scores
            # critical path in engine order (AT, k chunks, q chunks); the
            # Activation queue carries the rest (plain DMAs there are safe;
            # xbar transposes there returned corrupted data on HW).
            # Engine order on the serialized DMA pool = issue order: AT and
            # both k chunks first (they gate the kA groups), then q chunks
            # (consumed by the scores chains as they land).
            AT_v = AT16.ap().rearrange("(hpb p) h -> p hpb h", p=P)
            AT_s = acts.tile([P, HB, HIDDEN], F16, tag="AT")
            nc.sync.dma_start(out=AT_s[:, 0, :], in_=AT_v[:, 0, :])
            kTc = []
            for mch in range(M // 512):
                t = acts.tile([P, HB, 512], F16, tag=f"kT{mch}")
                nc.sync.dma_start(
                    out=t, in_=kT_v[:, :, mch * 512 : (mch + 1) * 512]
                )
                kTc.append(t)
            nc.sync.dma_start(out=AT_s[:, 1:, :], in_=AT_v[:, 1:, :])
            qTc = []
            for nch in range(NCH):
                t = acts.tile([P, HB, 512], F16, tag=f"qT{nch}")
                nc.sync.dma_start(
                    out=t, in_=qT_v[:, :, nch * 512 : (nch + 1) * 512]
                )
                qTc.append(t)
            # W and vT are consumed late (scores end / vpo); keep them on
            # the SAME queue BEHIND the critical chain — a second queue's
            # issues race ahead and steal the serialized DMA engine.
            W_s = acts.tile([P, HB, HIDDEN], F16, tag="W")
            nc.sync.dma_start(
                out=W_s, in_=W16.ap().rearrange("(hb p) h -> p hb h", p=P)
            )
            vT = acts.tile([P, HB, M], F16, tag="vT")
            nc.sync.dma_start(out=vT, in_=vT_v)
            if include_bvo:
                bvo_row = singles.tile([1, HIDDEN], F16)
                nc.scalar.dma_start(
                    out=bvo_row, in_=bvo16.ap().rearrange("(o a) -> o a", o=1)
                )
                ones_tmp = singles.tile([1, P], F16)
                nc.vector.memset(ones_tmp, 1.0)
                ones_row = singles.tile([1, P], F16)
                nc.vector.tensor_copy(ones_row, ones_tmp)
            rsum = singles.tile([P, MB], F32)
            rinv = singles.tile([P, MB], F32)

            kaT = acts.tile([P, HB, M], F16, tag="kaT")
            attnT = acts.tile([P, MB, N], BF16, tag="attnT")
            vpo_s = acts.tile([P, MB, HIDDEN], BF16, tag="vpo")

            with tc.tile_pool(name="ps", bufs=2, space="PSUM") as psp:
                # --- kaT[h, m] = sum_h' A[h, h'] k[m, h']  (lhsT = A^T) ---
                # The m-side of the bilinear score form is half the q-side,
                # so fold A into k rather than into q. Grouped by m-chunk:
                # one megatile holds all four h-blocks for half the keys, so
                # kaT[:, :, 0:512] (scores mb 0-3) completes after group 0.
                for mch in range(M // 512):
                    mega = psp.tile([P, N], F32, tag="mega", name=f"ka{mch}")
                    for hb in range(HB):
                        bank = mega[:, hb * 512 : (hb + 1) * 512]
                        for hpb in range(HB):
                            nc.tensor.matmul(
                                bank,
                                AT_s[:, hpb, hb * P : (hb + 1) * P],
                                kTc[mch][:, hpb, :],
                                start=(hpb == 0),
                                stop=(hpb == HB - 1),
                            )
                    for hb in range(HB):
                        if hb % 2 == 0:
                            nc.scalar.activation(
                                kaT[:, hb, mch * 512 : (mch + 1) * 512],
                                mega[:, hb * 512 : (hb + 1) * 512],
                                AF.Identity,
                            )
                        else:
                            nc.vector.tensor_copy(
                                kaT[:, hb, mch * 512 : (mch + 1) * 512],
                                mega[:, hb * 512 : (hb + 1) * 512],
                            )

                # --- scoresT + softmax(exp, unnormalized) per m-block ----
                # scoresT[m, n] = sum_h kaT[h, m] * qT[h, n]
                for mb in range(MB):
                    sc = psp.tile([P, N], F32, tag="mega", name=f"sc{mb}")
                    for hb in range(HB):
                        w = kaT[:, hb, mb * P : (mb + 1) * P]
                        for nch in range(NCH):
                            nc.tensor.matmul(
                                sc[:, nch * 512 : (nch + 1) * 512],
                                w,
                                qTc[nch][:, hb, :],
                                start=(hb == 0),
                                stop=(hb == HB - 1),
                            )
                    # e = exp(scores); rowsum via accumulator. No max-shift:
                    # |scores| < ~60 and exp(60) is comfortably finite in f32.
                    nc.scalar.activation(
                        attnT[:, mb, :],
                        sc,
                        AF.Exp,
                        accum_out=rsum[:, mb : mb + 1],
                    )
                    nc.vector.reciprocal(rinv[:, mb : mb + 1], rsum[:, mb : mb + 1])

                # --- vpo_s = (v @ Wvo + bvo) * rinv (normalization folded)
                # one megatile hosts FOUR [128,512] vpo banks, so the two
                # pool slots cover all 8 m-blocks with zero PSUM reuse stalls
                for quad in range(MB // 4):
                    mega = psp.tile([P, N], F32, tag="mega", name=f"vp{quad}")
                    for sub in range(4):
                        mb = 4 * quad + sub
                        ps = mega[:, sub * 512 : (sub + 1) * 512]
                        for hb in range(HB):
                            nc.tensor.matmul(
                                ps,
                                vT[:, hb, mb * P : (mb + 1) * P],
                                W_s[:, hb, :],
                                start=(hb == 0),
                                stop=(not include_bvo and hb == HB - 1),
                            )
                        if include_bvo:
                            nc.tensor.matmul(
                                ps, ones_row, bvo_row, start=False, stop=True
                            )
                        nc.scalar.activation(
                            vpo_s[:, mb, :],
                            ps,
                            AF.Identity,
                            scale=rinv[:, mb : mb + 1],
                        )

                # --- outT[h, n] = sum_m vpo_s[m, h] * attnT[m, n] --------
                for nch in range(NCH):
                    mega = psp.tile([P, N], F32, tag="mega", name=f"ob{nch}")
                    for mb in range(MB):
                        for hb in range(HB):
                            nc.tensor.matmul(
                                mega[:, hb * 512 : (hb + 1) * 512],
                                vpo_s[:, mb, hb * P : (hb + 1) * P],
                                attnT[:, mb, nch * 512 : (nch + 1) * 512],
                                start=(mb == 0),
                                stop=(mb == MB - 1),
                            )
                    # two half-DMAs per chunk: the first half's transfer
                    # overlaps the second half's PSUM->SBUF copies
                    st = ostg.tile([P, HB, 512], F16, tag="st")
                    for half in range(2):
                        for hb in (2 * half, 2 * half + 1):
                            if hb % 2 == 0:
                                nc.vector.tensor_copy(
                                    st[:, hb, :], mega[:, hb * 512 : (hb + 1) * 512]
                                )
                            else:
                                nc.scalar.activation(
                                    st[:, hb, :],
                                    mega[:, hb * 512 : (hb + 1) * 512],
                                    AF.Identity,
                                )
                        nc.sync.dma_start(
                            out=outT_v[
                                :,
                                2 * half : 2 * half + 2,
                                nch * 512 : (nch + 1) * 512,
                            ],
                            in_=st[:, 2 * half : 2 * half + 2, :],
                        )
    # A handful of waits are attached after the TileContext's own exit
    # processing; sweep again until the module is clean.
    while split_excess_waits(nc):
        pass
    return nc


class _Runner:
    """Compile the Bass program once; re-execute cheaply on later calls.

    Mirrors bass2jax.run_bass_via_pjrt's multi-core path, but keeps the
    jitted shard_map callable so repeated kernel() calls skip the
    multi-minute neuronxcc compile.
    """

    def __init__(self, include_bvo=False):
        import jax
        from jax.sharding import Mesh, PartitionSpec
        from jax.experimental.shard_map import shard_map
        from concourse import bass2jax
        import concourse.mybir as mb

        self.jax = jax
        nc = build_nc(include_bvo=include_bvo)
        self.nc = nc
        bass2jax.install_neuronx_cc_hook()

        in_names, out_names, out_avals, zero_outs = [], [], [], []
        partition_name = (
            nc.partition_id_tensor.name if nc.partition_id_tensor else None
        )
        for alloc in nc.m.functions[0].allocations:
            if not isinstance(alloc, mb.MemoryLocationSet):
                continue
            name = alloc.memorylocations[0].name
            if alloc.kind == "ExternalInput":
                if name != partition_name:
                    in_names.append(name)
            elif alloc.kind == "ExternalOutput":
                shape = tuple(alloc.tensor_shape)
                dtype = mb.dt.np(alloc.dtype)
                out_names.append(name)
                out_avals.append(jax.core.ShapedArray(shape, dtype))
                zero_outs.append(np.zeros(shape, dtype))
        n_params = len(in_names)
        n_outs = len(out_avals)
        all_in_names = list(in_names) + list(out_names)
        if partition_name is not None:
            all_in_names.append(partition_name)
        self.in_names = in_names
        self.out_names = out_names
        self.zero_outs = zero_outs

        def _body(*args):
            operands = list(args)
            if partition_name is not None:
                operands.append(bass2jax.partition_id_tensor())
            outs = bass2jax._bass_exec_p.bind(
                *operands,
                out_avals=tuple(out_avals),
                in_names=tuple(all_in_names),
                out_names=tuple(out_names),
                lowering_input_output_aliases=(),
                sim_require_finite=True,
                sim_require_nnan=True,
                nc=nc,
            )
            return tuple(outs)

        devices = jax.devices()[:8]
        mesh = Mesh(np.asarray(devices), ("core",))
        self.mesh = mesh
        in_specs = (PartitionSpec("core"),) * (n_params + n_outs)
        out_specs = (PartitionSpec("core"),) * n_outs
        self.body = _body
        self.in_specs = in_specs
        self.out_specs = out_specs
        donate = tuple(range(n_params, n_params + n_outs))
        self.sharded = jax.jit(
            shard_map(
                _body,
                mesh=mesh,
                in_specs=in_specs,
                out_specs=out_specs,
                check_rep=False,
            ),
            donate_argnums=donate,
            keep_unused=True,
        )
        self.out_avals = out_avals

    def prepare(self, in_maps):
        """Concatenate per-core inputs along axis 0 (device-shardable)."""
        return [
            np.concatenate([in_maps[c][name] for c in range(8)], axis=0)
            for name in self.in_names
        ]

    def run(self, concat_in):
        zeros = [
            np.zeros((8 * z.shape[0], *z.shape[1:]), z.dtype) for z in self.zero_outs
        ]
        out_arrs = self.sharded(*concat_in, *zeros)
        res = []
        for c in range(8):
            res.append(
                {
                    name: np.asarray(out_arrs[i]).reshape(
                        8, *self.out_avals[i].shape
                    )[c]
                    for i, name in enumerate(self.out_names)
                }
            )
        return res


_RUNNERS = {}


def _get_runner(include_bvo=False):
    if include_bvo not in _RUNNERS:
        _RUNNERS[include_bvo] = _Runner(include_bvo=include_bvo)
    return _RUNNERS[include_bvo]


def make_in_maps(inputs):
    f32 = lambda x: np.asarray(x, dtype=np.float32)
    q, k, v = f32(inputs["q"]), f32(inputs["k"]), f32(inputs["v"])
    Wq, Wk, Wv, Wo = (f32(inputs[n]) for n in ("Wq", "Wk", "Wv", "Wo"))
    bk, bv = f32(inputs["bk"]), f32(inputs["bv"])
    if np.any(bk != 0):
        # A nonzero bk adds a per-query term q @ (SCALE*Wq@bk) to the
        # scores which this kernel does not materialize. (bq-only terms
        # cancel exactly under the softmax over queries; bv/bo are handled.)
        raise NotImplementedError("nonzero bk is not supported by this kernel")
    AT16 = np.ascontiguousarray((SCALE * (Wk @ Wq.T)).astype(np.float16))
    W16 = np.ascontiguousarray((Wv @ Wo).astype(np.float16))
    bvo16 = np.ascontiguousarray((bv @ Wo).astype(np.float16))
    # host-side fp16 cast AND transpose: the device consumes q/k/v with the
    # hidden dim on partitions, so ship them pre-transposed and every load
    # becomes a plain contiguous-ish DMACopy (no xbar transposes).
    q16 = q.astype(np.float16)
    k16 = k.astype(np.float16)
    v16 = v.astype(np.float16)
    in_maps = []
    for c in range(8):
        b, mh = c // 2, c % 2
        sl = slice(mh * M, (mh + 1) * M)
        in_maps.append(
            {
                "qT16": np.ascontiguousarray(q16[b].T),
                "kT16": np.ascontiguousarray(k16[b, sl].T),
                "vT16": np.ascontiguousarray(v16[b, sl].T),
                "AT16": AT16,
                "W16": W16,
                "bvo16": bvo16,
            }
        )
    return in_maps


def assemble_out(results, bo):
    out = np.empty((B, N, HIDDEN), dtype=np.float32)
    for b in range(B):
        acc = results[2 * b]["outT"].astype(np.float32) + results[2 * b + 1]["outT"]
        out[b] = acc.T + bo[None, :]
    return out


def kernel(**inputs):
    in_maps = make_in_maps(inputs)
    include_bvo = bool(np.any(in_maps[0]["bvo16"] != 0))
    runner = _get_runner(include_bvo=include_bvo)
    res = runner.run(runner.prepare(in_maps))
    bo = np.asarray(inputs["bo"], dtype=np.float32)
    return assemble_out(res, bo)


# revision 48
# speedup vs baseline: 104.6302x; 79.8894x over previous
"""Trainium2 Bass kernel for nn_MultiHeadAttention_79706003079680.

Reference (fp32):
    qp = (q @ Wq + bq) * SCALE      # [B, N, PROJ]
    kp = k @ Wk + bk
    vp = v @ Wv + bv
    scores = einsum('bnd,bmd->bnm', qp, kp)
    attn = softmax(scores, axis=1)          # over the QUERY axis n
    x = einsum('bnm,bmd->bnd', attn, vp)
    out = x @ Wo + bo                       # [B, N, HIDDEN]

Algebraic collapse (the PROJ=4096 dim never materializes):
    scores = q @ A @ k^T (+ per-m terms that cancel under softmax over n,
                          + per-n term q @ (SCALE*Wq@bk) -- requires bk == 0,
                          asserted host-side; bq cancels EXACTLY for any bq)
      with A   = SCALE * Wq @ Wk^T          # [512, 512], host-precomputed
    out = attn @ (v @ Wvo + bv@Wo) + bo
      with Wvo = Wv @ Wo                    # [512, 512], host-precomputed

Per-core MACs drop ~10x vs the reference shape (51.5e9 -> 5.4e9 per batch).

Sharding: 8 cores = 4 batches x 2 key-halves (m in [mh*1024, mh*1024+1024)).
softmax(axis=n) is per-key-m, so an m-split needs no cross-core coupling.
Each core emits a partial out^T [HIDDEN, N]; the host sums the two
key-halves per batch, transposes, and adds bo (same as before).

On-chip flow (per core, P=128 partitions, everything SBUF-resident):
  qT  [128, 4, 2048] fp16   HOST-pre-transposed, plain DMA (4 chunk tiles)
  kT  [128, 4, 1024] fp16   HOST-pre-transposed, plain DMA (2 chunk tiles)
  vT  [128, 4, 1024] fp16   HOST-pre-transposed, plain DMA
  kaT = A^T k^T [128, 4, 1024] fp16  (the m-side of the bilinear form is
        half the size of the q-side, so associate scores = q (A k^T))
  scoresT[m, n] in PSUM megatiles [128, 2048] f32, contraction over h,
        lhsT = kaT blocks, rhs = raw qT chunks
  attnT = exp(scoresT) bf16 (UNNORMALIZED; rowsum via activation accum)
  vpo_s = (v @ Wvo + bvo) * (1/rowsum)  bf16  (per-partition scale = per-m)
  outT += vpo_s^T-blocks @ attnT        (normalization folded into vpo_s)

All load DMAs ride the SP (nc.sync) HWDGE queue in dependency-consumption
order (a second queue's issues race ahead on the serialized DMA engines;
xbar transposes on the Activation queue returned corrupted data on HW).
All PSUM phases allocate [128, 2048] megatiles from ONE pool (slices as
banks) so no pool-transition barriers serialize the phases.
"""

import numpy as np

import concourse.bass as bass
import concourse.mybir as mybir
import concourse.tile as tile

P = 128
HIDDEN = 512
B, N = 4, 2048
M = N // 2                         # keys per core = 1024
SCALE = (HIDDEN // 8) ** -0.5      # att_size ** -0.5 = 64 ** -0.5

HB = HIDDEN // P                   # 4 h-blocks of 128
NCH = N // 512                     # 4 n-chunks of 512
MB = M // P                        # 8 m-blocks of 128

F32 = mybir.dt.float32
F16 = mybir.dt.float16
BF16 = mybir.dt.bfloat16
AF = mybir.ActivationFunctionType


MAX_WAITS = 1


def split_excess_waits(nc, max_waits=MAX_WAITS):
    """Move excess per-instruction sem waits onto same-engine NoOps.

    This walrus build rejects instructions carrying more than a couple of
    sync-wait commands ("Too many sync wait commands" in setupSyncWait).
    A NoOp placed immediately before the instruction on the same engine
    enforces the wait in program order with identical semantics.
    """
    n_extra = 0
    for f in nc.m.functions:
        for bb in f.blocks:
            insts = bb.instructions
            i = 0
            while i < len(insts):
                inst = insts[i]
                si = getattr(inst, "sync_info", None)
                if si is not None and si.on_wait and len(si.on_wait) > max_waits:
                    waits = list(si.on_wait)
                    si.on_wait = waits[: max_waits]
                    for w in waits[max_waits:]:
                        n_extra += 1
                        nop = mybir.InstNoOp(
                            name=f"I-wsplit{n_extra}",
                            ins=[],
                            outs=[],
                            engine=inst.engine,
                        )
                        nop.sync_info = mybir.SyncInfo(on_wait=[w], on_update=[])
                        try:
                            nc.register_instruction(nop)
                        except Exception:
                            pass
                        # insert immediately before inst (inst shifts right)
                        insts.insert(i, nop)
                        i += 1
                i += 1
    return n_extra


class PatchedTC(tile.TileContext):
    """TileContext that post-processes the module to satisfy this walrus
    build's per-instruction sync-wait limit."""

    def __exit__(self, exc_type, exc_val, exc_tb):
        ret = super().__exit__(exc_type, exc_val, exc_tb)
        if exc_type is None:
            split_excess_waits(self.nc)
        return ret


def build_nc(include_bvo=False):
    nc = bass.Bass("TRN2", target_bir_lowering=False, debug=False, num_devices=8)

    qT16 = nc.dram_tensor("qT16", [HIDDEN, N], F16, kind="ExternalInput")
    kT16 = nc.dram_tensor("kT16", [HIDDEN, M], F16, kind="ExternalInput")
    vT16 = nc.dram_tensor("vT16", [HIDDEN, M], F16, kind="ExternalInput")
    AT16 = nc.dram_tensor("AT16", [HIDDEN, HIDDEN], F16, kind="ExternalInput")
    W16 = nc.dram_tensor("W16", [HIDDEN, HIDDEN], F16, kind="ExternalInput")
    if include_bvo:
        bvo16 = nc.dram_tensor("bvo16", [HIDDEN], F16, kind="ExternalInput")
    outT = nc.dram_tensor("outT", [HIDDEN, N], F16, kind="ExternalOutput")

    qT_v = qT16.ap().rearrange("(hb p) n -> p hb n", p=P)
    kT_v = kT16.ap().rearrange("(hb p) m -> p hb m", p=P)
    vT_v = vT16.ap().rearrange("(hb p) m -> p hb m", p=P)

    outT_v = outT.ap().rearrange("(hb p) n -> p hb n", p=P)

    with PatchedTC(nc) as tc:
        with (
            tc.tile_pool(name="singles", bufs=1) as singles,
            tc.tile_pool(name="acts", bufs=1) as acts,
            tc.tile_pool(name="ostg", bufs=2) as ostg,
        ):
            # --- loads. q/k/v arrive HOST-pre-transposed, so everything is
            # a plain DMACopy. All loads ride the SP (nc.sync) HWDGE queue:
            # engine order on the serialized DMA pool follows issue order,
            # and a second queue's issues would race ahead and steal the
            # engine (also, xbar transposes on the Activation queue returned
            # corrupted data on HW). Order: the first AT quarter and both k
            # chunks (they gate the kA groups), then q chunks (consumed by
            # the scores chains as they land), then late-consumed W/vT.
            # AT quarter 0 + k chunk 0 gate the first accumulation chain;
            # the AT remainder must land before the chain's second step, so
            # it is issued between the two k chunks.
            AT_v = AT16.ap().rearrange("(hpb p) h -> p hpb h", p=P)
            AT_s = acts.tile([P, HB, HIDDEN], F16, tag="AT")
            nc.sync.dma_start(out=AT_s[:, 0, :], in_=AT_v[:, 0, :])
            kTc = []
            t0 = acts.tile([P, HB, 512], F16, tag="kT0")
            nc.sync.dma_start(out=t0, in_=kT_v[:, :, 0:512])
            kTc.append(t0)
            nc.sync.dma_start(out=AT_s[:, 1:, :], in_=AT_v[:, 1:, :])
            t1 = acts.tile([P, HB, 512], F16, tag="kT1")
            nc.sync.dma_start(out=t1, in_=kT_v[:, :, 512:M])
            kTc.append(t1)
            qTc = []
            for nch in range(NCH):
                t = acts.tile([P, HB, 512], F16, tag=f"qT{nch}")
                nc.sync.dma_start(
                    out=t, in_=qT_v[:, :, nch * 512 : (nch + 1) * 512]
                )
                qTc.append(t)
            # W and vT are consumed late (scores end / vpo); keep them on
            # the SAME queue BEHIND the critical chain — a second queue's
            # issues race ahead and steal the serialized DMA engine.
            W_s = acts.tile([P, HB, HIDDEN], F16, tag="W")
            nc.sync.dma_start(
                out=W_s, in_=W16.ap().rearrange("(hb p) h -> p hb h", p=P)
            )
            vT = acts.tile([P, HB, M], F16, tag="vT")
            nc.sync.dma_start(out=vT, in_=vT_v)
            if include_bvo:
                bvo_row = singles.tile([1, HIDDEN], F16)
                nc.scalar.dma_start(
                    out=bvo_row, in_=bvo16.ap().rearrange("(o a) -> o a", o=1)
                )
                ones_tmp = singles.tile([1, P], F16)
                nc.vector.memset(ones_tmp, 1.0)
                ones_row = singles.tile([1, P], F16)
                nc.vector.tensor_copy(ones_row, ones_tmp)
            rsum = singles.tile([P, MB], F32)
            rinv = singles.tile([P, MB], F32)

            kaT = acts.tile([P, HB, M], F16, tag="kaT")
            attnT = acts.tile([P, MB, N], BF16, tag="attnT")
            vpo_s = acts.tile([P, MB, HIDDEN], BF16, tag="vpo")

            with tc.tile_pool(name="ps", bufs=2, space="PSUM") as psp:
                # --- kaT[h, m] = sum_h' A[h, h'] k[m, h']  (lhsT = A^T) ---
                # The m-side of the bilinear score form is half the q-side,
                # so fold A into k rather than into q. Grouped by m-chunk:
                # one megatile holds all four h-blocks for half the keys, so
                # kaT[:, :, 0:512] (scores mb 0-3) completes after group 0.
                for mch in range(M // 512):
                    mega = psp.tile([P, N], F32, tag="mega", name=f"ka{mch}")
                    for hb in range(HB):
                        bank = mega[:, hb * 512 : (hb + 1) * 512]
                        for hpb in range(HB):
                            nc.tensor.matmul(
                                bank,
                                AT_s[:, hpb, hb * P : (hb + 1) * P],
                                kTc[mch][:, hpb, :],
                                start=(hpb == 0),
                                stop=(hpb == HB - 1),
                            )
                    for hb in range(HB):
                        if hb % 2 == 0:
                            nc.scalar.activation(
                                kaT[:, hb, mch * 512 : (mch + 1) * 512],
                                mega[:, hb * 512 : (hb + 1) * 512],
                                AF.Identity,
                            )
                        else:
                            nc.vector.tensor_copy(
                                kaT[:, hb, mch * 512 : (mch + 1) * 512],
                                mega[:, hb * 512 : (hb + 1) * 512],
                            )

                # --- scoresT + softmax(exp, unnormalized) per m-block ----
                # scoresT[m, n] = sum_h kaT[h, m] * qT[h, n]
                for mb in range(MB):
                    sc = psp.tile([P, N], F32, tag="mega", name=f"sc{mb}")
                    for hb in range(HB):
                        w = kaT[:, hb, mb * P : (mb + 1) * P]
                        for nch in range(NCH):
                            nc.tensor.matmul(
                                sc[:, nch * 512 : (nch + 1) * 512],
                                w,
                                qTc[nch][:, hb, :],
                                start=(hb == 0),
                                stop=(hb == HB - 1),
                            )
                    # e = exp(scores); rowsum via accumulator. No max-shift:
                    # |scores| < ~60 and exp(60) is comfortably finite in f32.
                    nc.scalar.activation(
                        attnT[:, mb, :],
                        sc,
                        AF.Exp,
                        accum_out=rsum[:, mb : mb + 1],
                    )
                    nc.vector.reciprocal(rinv[:, mb : mb + 1], rsum[:, mb : mb + 1])

                # --- vpo_s = (v @ Wvo + bvo) * rinv (normalization folded)
                # one megatile hosts FOUR [128,512] vpo banks, so the two
                # pool slots cover all 8 m-blocks with zero PSUM reuse stalls
                for quad in range(MB // 4):
                    mega = psp.tile([P, N], F32, tag="mega", name=f"vp{quad}")
                    for sub in range(4):
                        mb = 4 * quad + sub
                        ps = mega[:, sub * 512 : (sub + 1) * 512]
                        for hb in range(HB):
                            nc.tensor.matmul(
                                ps,
                                vT[:, hb, mb * P : (mb + 1) * P],
                                W_s[:, hb, :],
                                start=(hb == 0),
                                stop=(not include_bvo and hb == HB - 1),
                            )
                        if include_bvo:
                            nc.tensor.matmul(
                                ps, ones_row, bvo_row, start=False, stop=True
                            )
                        nc.scalar.activation(
                            vpo_s[:, mb, :],
                            ps,
                            AF.Identity,
                            scale=rinv[:, mb : mb + 1],
                        )

                # --- outT[h, n] = sum_m vpo_s[m, h] * attnT[m, n] --------
                for nch in range(NCH):
                    mega = psp.tile([P, N], F32, tag="mega", name=f"ob{nch}")
                    for mb in range(MB):
                        for hb in range(HB):
                            nc.tensor.matmul(
                                mega[:, hb * 512 : (hb + 1) * 512],
                                vpo_s[:, mb, hb * P : (hb + 1) * P],
                                attnT[:, mb, nch * 512 : (nch + 1) * 512],
                                start=(mb == 0),
                                stop=(mb == MB - 1),
                            )
                    # two half-DMAs per chunk: the first half's transfer
                    # overlaps the second half's PSUM->SBUF copies
                    st = ostg.tile([P, HB, 512], F16, tag="st")
                    for half in range(2):
                        for hb in (2 * half, 2 * half + 1):
                            if nch == NCH - 1 or hb % 2 == 0:
                                nc.vector.tensor_copy(
                                    st[:, hb, :], mega[:, hb * 512 : (hb + 1) * 512]
                                )
                            else:
                                nc.scalar.activation(
                                    st[:, hb, :],
                                    mega[:, hb * 512 : (hb + 1) * 512],
                                    AF.Identity,
                                )
                        nc.sync.dma_start(
                            out=outT_v[
                                :,
                                2 * half : 2 * half + 2,
                                nch * 512 : (nch + 1) * 512,
                            ],
                            in_=st[:, 2 * half : 2 * half + 2, :],
                        )
    # A handful of waits are attached after the TileContext's own exit
    # processing; sweep again until the module is clean.
    while split_excess_waits(nc):
        pass
    return nc


class _Runner:
    """Compile the Bass program once; re-execute cheaply on later calls.

    Mirrors bass2jax.run_bass_via_pjrt's multi-core path, but keeps the
    jitted shard_map callable so repeated kernel() calls skip the
    multi-minute neuronxcc compile.
    """

    def __init__(self, include_bvo=False):
        import jax
        from jax.sharding import Mesh, PartitionSpec
        from jax.experimental.shard_map import shard_map
        from concourse import bass2jax
        import concourse.mybir as mb

        self.jax = jax
        nc = build_nc(include_bvo=include_bvo)
        self.nc = nc
        bass2jax.install_neuronx_cc_hook()

        in_names, out_names, out_avals, zero_outs = [], [], [], []
        partition_name = (
            nc.partition_id_tensor.name if nc.partition_id_tensor else None
        )
        for alloc in nc.m.functions[0].allocations:
            if not isinstance(alloc, mb.MemoryLocationSet):
                continue
            name = alloc.memorylocations[0].name
            if alloc.kind == "ExternalInput":
                if name != partition_name:
                    in_names.append(name)
            elif alloc.kind == "ExternalOutput":
                shape = tuple(alloc.tensor_shape)
                dtype = mb.dt.np(alloc.dtype)
                out_names.append(name)
                out_avals.append(jax.core.ShapedArray(shape, dtype))
                zero_outs.append(np.zeros(shape, dtype))
        n_params = len(in_names)
        n_outs = len(out_avals)
        all_in_names = list(in_names) + list(out_names)
        if partition_name is not None:
            all_in_names.append(partition_name)
        self.in_names = in_names
        self.out_names = out_names
        self.zero_outs = zero_outs

        def _body(*args):
            operands = list(args)
            if partition_name is not None:
                operands.append(bass2jax.partition_id_tensor())
            outs = bass2jax._bass_exec_p.bind(
                *operands,
                out_avals=tuple(out_avals),
                in_names=tuple(all_in_names),
                out_names=tuple(out_names),
                lowering_input_output_aliases=(),
                sim_require_finite=True,
                sim_require_nnan=True,
                nc=nc,
            )
            return tuple(outs)

        devices = jax.devices()[:8]
        mesh = Mesh(np.asarray(devices), ("core",))
        self.mesh = mesh
        in_specs = (PartitionSpec("core"),) * (n_params + n_outs)
        out_specs = (PartitionSpec("core"),) * n_outs
        self.body = _body
        self.in_specs = in_specs
        self.out_specs = out_specs
        donate = tuple(range(n_params, n_params + n_outs))
        self.sharded = jax.jit(
            shard_map(
                _body,
                mesh=mesh,
                in_specs=in_specs,
                out_specs=out_specs,
                check_rep=False,
            ),
            donate_argnums=donate,
            keep_unused=True,
        )
        self.out_avals = out_avals

    def prepare(self, in_maps):
        """Concatenate per-core inputs along axis 0 (device-shardable)."""
        return [
            np.concatenate([in_maps[c][name] for c in range(8)], axis=0)
            for name in self.in_names
        ]

    def run(self, concat_in):
        zeros = [
            np.zeros((8 * z.shape[0], *z.shape[1:]), z.dtype) for z in self.zero_outs
        ]
        out_arrs = self.sharded(*concat_in, *zeros)
        res = []
        for c in range(8):
            res.append(
                {
                    name: np.asarray(out_arrs[i]).reshape(
                        8, *self.out_avals[i].shape
                    )[c]
                    for i, name in enumerate(self.out_names)
                }
            )
        return res


_RUNNERS = {}


def _get_runner(include_bvo=False):
    if include_bvo not in _RUNNERS:
        _RUNNERS[include_bvo] = _Runner(include_bvo=include_bvo)
    return _RUNNERS[include_bvo]


def make_in_maps(inputs):
    f32 = lambda x: np.asarray(x, dtype=np.float32)
    q, k, v = f32(inputs["q"]), f32(inputs["k"]), f32(inputs["v"])
    Wq, Wk, Wv, Wo = (f32(inputs[n]) for n in ("Wq", "Wk", "Wv", "Wo"))
    bk, bv = f32(inputs["bk"]), f32(inputs["bv"])
    if np.any(bk != 0):
        # A nonzero bk adds a per-query term q @ (SCALE*Wq@bk) to the
        # scores which this kernel does not materialize. (bq-only terms
        # cancel exactly under the softmax over queries; bv/bo are handled.)
        raise NotImplementedError("nonzero bk is not supported by this kernel")
    AT16 = np.ascontiguousarray((SCALE * (Wk @ Wq.T)).astype(np.float16))
    W16 = np.ascontiguousarray((Wv @ Wo).astype(np.float16))
    bvo16 = np.ascontiguousarray((bv @ Wo).astype(np.float16))
    # host-side fp16 cast AND transpose: the device consumes q/k/v with the
    # hidden dim on partitions, so ship them pre-transposed and every load
    # becomes a plain contiguous-ish DMACopy (no xbar transposes).
    q16 = q.astype(np.float16)
    k16 = k.astype(np.float16)
    v16 = v.astype(np.float16)
    in_maps = []
    for c in range(8):
        b, mh = c // 2, c % 2
        sl = slice(mh * M, (mh + 1) * M)
        in_maps.append(
            {
                "qT16": np.ascontiguousarray(q16[b].T),
                "kT16": np.ascontiguousarray(k16[b, sl].T),
                "vT16": np.ascontiguousarray(v16[b, sl].T),
                "AT16": AT16,
                "W16": W16,
                "bvo16": bvo16,
            }
        )
    return in_maps


def assemble_out(results, bo):
    out = np.empty((B, N, HIDDEN), dtype=np.float32)
    for b in range(B):
        acc = results[2 * b]["outT"].astype(np.float32) + results[2 * b + 1]["outT"]
        out[b] = acc.T + bo[None, :]
    return out


def kernel(**inputs):
    in_maps = make_in_maps(inputs)
    include_bvo = bool(np.any(in_maps[0]["bvo16"] != 0))
    runner = _get_runner(include_bvo=include_bvo)
    res = runner.run(runner.prepare(in_maps))
    bo = np.asarray(inputs["bo"], dtype=np.float32)
    return assemble_out(res, bo)
